# revision 1
# baseline (speedup 1.0000x reference)
"""Trainium2 (8 NeuronCores) kernel for coverage attention.

Computes, for inputs (B,S,H)=(64,2048,512):
    enc_f = encoder_output @ Wh + bh            [B,S,H]
    dec_f = decoder_hidden @ Ws + bs            [B,1,H]
    cov_f = coverage[...,None] * Wc[0] + bc     [B,S,H]
    feat  = tanh(enc_f + dec_f + cov_f)
    e_t   = feat @ v_w + v_b                    [B,S]
    a_t   = softmax(e_t, axis=-1)
    sum_coverage = coverage + a_t
returns (a_t, sum_coverage).

Sharding: data-parallel over batch B across 8 cores (8 batches/core).
Params are small and replicated. No collectives needed.

Per-core pipeline (hardware-verified structure):
  - X tiles [128 s, 512 h] are PE-transposed (fp32) into PSUM, then
    DVE-copied (rounding to float32r) as Xt_k [128 h, 512 s] in SBUF.
  - feat.T chunks [h=128, s=512] = Wh_k.T @ Xt_k accumulated in PSUM with
    float32r operands (1 cycle/row on TensorE); the coverage term enters
    the same accumulation as a K=1 matmul  Wc[ms].T (x) cov[gs].
  - bias A[b,h] = dec_f + bh + bs + bc is applied via the tanh
    activation's per-partition bias (A computed on device with f32r
    matmuls; biases folded in via a rank-1 ones matmul).
  - e_t row chunks [1, 512] = v_w.T @ feat chunks (K=128, M=1 matmuls).
  - per-batch e rows are gathered into [8, 2048] via SB->SB DMA;
    softmax uses free-dim reduce + Exp(bias=-max, accum_out=sum).
  - v_b is omitted: softmax is invariant to constant shifts.
"""

import os
import sys

for _p in ("/opt/trn_rl_repo", os.path.expanduser("~/.axon_site/_ro/trn_rl_repo")):
    if os.path.isdir(_p) and _p not in sys.path:
        sys.path.insert(0, _p)

import numpy as np

import concourse.bass as bass
from concourse import bacc
import concourse.tile as tile
from concourse import mybir
from concourse.masks import make_identity

B, S, H = 64, 2048, 512
N_CORES = 8
BPC = B // N_CORES  # batches per core

FP = mybir.dt.float32
FPR = mybir.dt.float32r

GROUP = 512          # seq positions processed per inner group
HC = H // 128        # h chunks of 128


def build_program(bpc=BPC, s=S, mm_fast=True):
    """Build the per-core Bass program. mm_fast=True uses float32r matmuls."""
    nc = bacc.Bacc(None)
    n_groups = s // GROUP
    it = GROUP // 128  # 128-row seq tiles per group

    WDT = FPR if mm_fast else FP  # dtype for matmul operands

    x = nc.declare_dram_parameter("x", [bpc * s, H], FP, isOutput=False)
    dh = nc.declare_dram_parameter("dh", [bpc, H], FP, isOutput=False)
    cov = nc.declare_dram_parameter("cov", [bpc, s], FP, isOutput=False)
    cov2 = nc.declare_dram_parameter("cov2", [1, bpc * s], WDT, isOutput=False)
    wh = nc.declare_dram_parameter("wh", [H, H], WDT, isOutput=False)
    ws = nc.declare_dram_parameter("ws", [H, H], WDT, isOutput=False)
    wc = nc.declare_dram_parameter("wc", [1, H], WDT, isOutput=False)
    vw = nc.declare_dram_parameter("vw", [1, H], FP, isOutput=False)
    bh = nc.declare_dram_parameter("bh", [1, H], WDT, isOutput=False)
    bs = nc.declare_dram_parameter("bs", [1, H], WDT, isOutput=False)
    bc = nc.declare_dram_parameter("bc", [1, H], WDT, isOutput=False)
    ones = nc.declare_dram_parameter("ones", [1, bpc], WDT, isOutput=False)
    out_a = nc.declare_dram_parameter("out_a", [bpc, s], FP, isOutput=True)
    out_sc = nc.declare_dram_parameter("out_sc", [bpc, s], FP, isOutput=True)

    from contextlib import ExitStack
    with tile.TileContext(nc) as tc, ExitStack() as ctx:
        const = ctx.enter_context(tc.tile_pool(name="const", bufs=1))
        xpool = ctx.enter_context(tc.tile_pool(name="xpool", bufs=3))
        xtpool = ctx.enter_context(tc.tile_pool(name="xtpool", bufs=8))
        fpool = ctx.enter_context(tc.tile_pool(name="fpool", bufs=3))
        covg_pool = ctx.enter_context(tc.tile_pool(name="covg", bufs=3))
        erow_pool = ctx.enter_context(tc.tile_pool(name="erow", bufs=3))
        ps_xt_pool = ctx.enter_context(tc.tile_pool(name="ps_xt", bufs=2, space="PSUM"))
        ps_f_pool = ctx.enter_context(tc.tile_pool(name="ps_f", bufs=2, space="PSUM"))
        ps_e_pool = ctx.enter_context(tc.tile_pool(name="ps_e", bufs=2, space="PSUM"))
        ps_pre_pool = ctx.enter_context(tc.tile_pool(name="ps_pre", bufs=1, space="PSUM"))

        # ---------------- preamble: constants & params ----------------
        ident = const.tile([128, 128], FP, tag="ident")
        make_identity(nc, ident)

        wh_sb = []
        ws_sb = []
        for k in range(HC):
            t = const.tile([128, H], WDT, tag=f"wh{k}", name=f"wh_sb{k}")
            nc.sync.dma_start(out=t, in_=wh[k * 128:(k + 1) * 128, :])
            wh_sb.append(t)
            t = const.tile([128, H], WDT, tag=f"ws{k}", name=f"ws_sb{k}")
            nc.sync.dma_start(out=t, in_=ws[k * 128:(k + 1) * 128, :])
            ws_sb.append(t)

        wc_sb = const.tile([1, H], WDT, tag="wc")
        nc.sync.dma_start(out=wc_sb, in_=wc[:, :])
        vw_row = const.tile([1, H], FP, tag="vw_row")
        nc.sync.dma_start(out=vw_row, in_=vw[:, :])
        bh_sb = const.tile([1, H], WDT, tag="bh")
        nc.sync.dma_start(out=bh_sb, in_=bh[:, :])
        bs_sb = const.tile([1, H], WDT, tag="bs")
        nc.sync.dma_start(out=bs_sb, in_=bs[:, :])
        bc_sb = const.tile([1, H], WDT, tag="bc")
        nc.sync.dma_start(out=bc_sb, in_=bc[:, :])
        dh_sb = const.tile([bpc, H], FP, tag="dh")
        nc.sync.dma_start(out=dh_sb, in_=dh[:, :])
        cov_sb = const.tile([bpc, s], FP, tag="cov")
        nc.sync.dma_start(out=cov_sb, in_=cov[:, :])

        # bias sum bh + bs + bc -> [1, H]
        bsum_sb = const.tile([1, H], WDT, tag="bsum")
        nc.vector.tensor_add(bsum_sb, bh_sb, bs_sb)
        nc.vector.tensor_add(bsum_sb, bsum_sb, bc_sb)

        ones_sb = const.tile([1, bpc], WDT, tag="ones")
        nc.sync.dma_start(out=ones_sb, in_=ones[:, :])

        # v_w chunked to [128, HC] via PE transpose of [1,128] slices
        vw_sb = const.tile([128, HC], WDT, tag="vw_sb")
        for k in range(HC):
            ps = ps_pre_pool.tile([128, max(bpc, 8)], FP, tag="pre")
            nc.tensor.transpose(
                ps[:, 0:1],
                vw_row[0:1, k * 128:(k + 1) * 128],
                ident[0:1, 0:1],
            )
            nc.vector.tensor_copy(vw_sb[:, k:k + 1], ps[:, 0:1])

        # decoder_hidden transposed: dhT_k [128, bpc]
        dht_sb = []
        for k in range(HC):
            ps = ps_pre_pool.tile([128, max(bpc, 8)], FP, tag="pre")
            nc.tensor.transpose(
                ps[:, 0:bpc],
                dh_sb[0:bpc, k * 128:(k + 1) * 128],
                ident[0:bpc, 0:bpc],
            )
            t = const.tile([128, bpc], WDT, tag=f"dht{k}", name=f"dht{k}")
            nc.vector.tensor_copy(t, ps[:, 0:bpc])
            dht_sb.append(t)

        # A[h, b] = (dh @ Ws).T + (bh + bs + bc) broadcast over b,
        # computed chunk-wise: psA_m = sum_k Ws[k,m].T @ dhT_k + bsum_m.T @ ones
        a_sb = const.tile([128, HC, bpc], FP, tag="a_sb")
        for m in range(HC):
            ms = slice(m * 128, (m + 1) * 128)
            ps = ps_pre_pool.tile([128, max(bpc, 8)], FP, tag="pre")
            for k in range(HC):
                nc.tensor.matmul(
                    ps[:, 0:bpc],
                    ws_sb[k][:, ms],
                    dht_sb[k][:, :],
                    start=(k == 0),
                    stop=False,
                )
            nc.tensor.matmul(
                ps[:, 0:bpc],
                bsum_sb[0:1, ms],
                ones_sb[0:1, :],
                start=False,
                stop=True,
            )
            nc.vector.tensor_copy(a_sb[:, m, :], ps[:, 0:bpc])

        # e_t accumulator [bpc, s]; per-group chunks go through a small
        # partition-0 scratch (DVE writes must start at partition 0) and a
        # SB->SB DMA into row b.
        e_sb = const.tile([bpc, s], FP, tag="e_sb")

        # ---------------- main loop ----------------
        for b in range(bpc):
            for g in range(n_groups):
                gs = slice(g * GROUP, (g + 1) * GROUP)
                r0 = b * s + g * GROUP
                x_g = x[r0:r0 + GROUP, :].rearrange("(i p) h -> p i h", p=128)
                x_in = xpool.tile([128, it, H], FP, tag="x_in")
                nc.sync.dma_start(out=x_in, in_=x_g)
                cov_g = covg_pool.tile([1, GROUP], WDT, tag="cov_g")
                nc.gpsimd.dma_start(
                    out=cov_g,
                    in_=cov2[0:1, b * s + g * GROUP:b * s + (g + 1) * GROUP])

                xts = []
                for k in range(HC):
                    ks = slice(k * 128, (k + 1) * 128)
                    ps_xt = ps_xt_pool.tile([128, GROUP], FP, tag="ps_xt")
                    for i in range(it):
                        nc.tensor.transpose(
                            ps_xt[:, i * 128:(i + 1) * 128],
                            x_in[:, i, ks],
                            ident[:, :],
                        )
                    xt_k = xtpool.tile([128, GROUP], WDT, tag="xt")
                    nc.vector.tensor_copy(xt_k, ps_xt)
                    xts.append(xt_k)

                ps_e = ps_e_pool.tile([1, GROUP], FP, tag="ps_e")
                for m in range(HC):
                    ms = slice(m * 128, (m + 1) * 128)
                    ps_f = ps_f_pool.tile([128, GROUP], FP, tag="ps_f")
                    for k in range(HC):
                        nc.tensor.matmul(
                            ps_f,
                            wh_sb[k][:, ms],
                            xts[k][:, :],
                            start=(k == 0),
                            stop=False,
                        )
                    nc.tensor.matmul(
                        ps_f,
                        wc_sb[0:1, ms],
                        cov_g[0:1, :],
                        start=False,
                        stop=True,
                    )
                    f_m = fpool.tile([128, GROUP], WDT, tag="f_m")
                    nc.scalar.activation(
                        out=f_m,
                        in_=ps_f,
                        func=mybir.ActivationFunctionType.Tanh,
                        bias=a_sb[:, m, b:b + 1],
                    )
                    nc.tensor.matmul(
                        ps_e,
                        vw_sb[:, m:m + 1],
                        f_m[:, :],
                        start=(m == 0),
                        stop=(m == HC - 1),
                    )
                e_g = erow_pool.tile([1, GROUP], FP, tag="e_g")
                nc.vector.tensor_copy(e_g, ps_e)
                nc.sync.dma_start(out=e_sb[b:b + 1, gs], in_=e_g)

        # ---------------- softmax + outputs ----------------
        smx = const.tile([bpc, 1], FP, tag="smx")
        nc.vector.tensor_reduce(
            out=smx, in_=e_sb, axis=mybir.AxisListType.X,
            op=mybir.AluOpType.max, negate=True,
        )
        p_sb = const.tile([bpc, s], FP, tag="p_sb")
        esum = const.tile([bpc, 1], FP, tag="esum")
        nc.scalar.activation(
            out=p_sb, in_=e_sb, func=mybir.ActivationFunctionType.Exp,
            bias=smx, accum_out=esum,
        )
        rsum = const.tile([bpc, 1], FP, tag="rsum")
        nc.vector.reciprocal(rsum, esum)
        a_out_sb = const.tile([bpc, s], FP, tag="a_out")
        nc.vector.tensor_scalar_mul(a_out_sb, p_sb, rsum)
        sc_sb = const.tile([bpc, s], FP, tag="sc_sb")
        nc.vector.tensor_add(sc_sb, a_out_sb, cov_sb)
        nc.sync.dma_start(out=out_a[:, :], in_=a_out_sb)
        nc.sync.dma_start(out=out_sc[:, :], in_=sc_sb)

    return nc


_PROG_CACHE = {}


def _get_program(key=(BPC, S, True)):
    if key not in _PROG_CACHE:
        nc = build_program(*key)
        nc.finalize()
        _PROG_CACHE[key] = nc
    return _PROG_CACHE[key]


def make_in_maps(encoder_output, decoder_hidden, coverage, Wh, bh, Ws, bs, Wc, bc,
                 v_w, v_b=None):
    f32 = np.float32
    enc = np.ascontiguousarray(encoder_output, dtype=f32)
    dh = np.ascontiguousarray(decoder_hidden, dtype=f32)
    cov = np.ascontiguousarray(coverage, dtype=f32)
    shared = {
        "wh": np.ascontiguousarray(Wh, dtype=f32),
        "ws": np.ascontiguousarray(Ws, dtype=f32),
        "wc": np.ascontiguousarray(Wc, dtype=f32).reshape(1, H),
        "vw": np.ascontiguousarray(v_w, dtype=f32).reshape(1, H),
        "bh": np.ascontiguousarray(bh, dtype=f32).reshape(1, H),
        "bs": np.ascontiguousarray(bs, dtype=f32).reshape(1, H),
        "bc": np.ascontiguousarray(bc, dtype=f32).reshape(1, H),
        "ones": np.ones((1, BPC), dtype=f32),
    }
    in_maps = []
    for c in range(N_CORES):
        lo, hi = c * BPC, (c + 1) * BPC
        m = dict(shared)
        m["x"] = np.ascontiguousarray(enc[lo:hi].reshape(BPC * S, H))
        m["dh"] = np.ascontiguousarray(dh[lo:hi])
        m["cov"] = np.ascontiguousarray(cov[lo:hi])
        m["cov2"] = m["cov"].reshape(1, -1)
        in_maps.append(m)
    return in_maps


def run_spmd(in_maps, trace=False, **kw):
    from concourse.bass_utils import run_bass_kernel_spmd
    nc = _get_program()
    return run_bass_kernel_spmd(nc, in_maps, core_ids=list(range(N_CORES)),
                                trace=trace, **kw)


def kernel(**inputs) -> tuple[np.ndarray, np.ndarray]:
    in_maps = make_in_maps(**inputs)
    res = run_spmd(in_maps)
    a_t = np.concatenate([r["out_a"] for r in res.results], axis=0)
    sum_cov = np.concatenate([r["out_sc"] for r in res.results], axis=0)
    return a_t.astype(np.float32), sum_cov.astype(np.float32)



# revision 4
# speedup vs baseline: 1.3013x; 1.3013x over previous
"""Trainium2 (8 NeuronCores) kernel for coverage attention.

Computes, for inputs (B,S,H)=(64,2048,512):
    enc_f = encoder_output @ Wh + bh            [B,S,H]
    dec_f = decoder_hidden @ Ws + bs            [B,1,H]
    cov_f = coverage[...,None] * Wc[0] + bc     [B,S,H]
    feat  = tanh(enc_f + dec_f + cov_f)
    e_t   = feat @ v_w + v_b                    [B,S]
    a_t   = softmax(e_t, axis=-1)
    sum_coverage = coverage + a_t
returns (a_t, sum_coverage).

Sharding: data-parallel over batch B across 8 cores (8 batches/core).
Params are small and replicated. No collectives needed.

Per-core pipeline (v3):
  - encoder_output is cast to fp16 and pre-transposed on the host into
    xT [H, bpc*s]; the device streams contiguous fp16 Xt tiles
    [128 h, 512 s] straight from HBM (half the f32 traffic, no PE
    transposes, no on-device casts).
  - feat.T chunks [h=128, s=512] = Wh_k.T @ Xt_k accumulated in fp32
    PSUM with fp16 operands (2 cols/cycle stream + FWL weight loads).
  - the coverage term enters the same PSUM accumulation as a K=1 fp16
    matmul  Wc16[ms].T (x) cov16[gs]  (~107 ns at 2 cols/cycle).
  - bias A[b,h] = dec_f + bh + bs + bc is applied via the tanh
    activation's per-partition bias; tanh emits fp16 f_m.
  - e_t row chunks [1, 512] = v_w.T @ f_m (K=128, M=1, fp16), emitted
    1-2 main blocks late so the in-order PE queue never waits on tanh.
  - per-batch e rows are gathered into [8, 2048] via SB->SB DMA;
    softmax uses free-dim reduce + Exp(bias=-max, accum_out=sum).
  - v_b is omitted: softmax is invariant to constant shifts.
"""

import os
import sys

for _p in ("/opt/trn_rl_repo", os.path.expanduser("~/.axon_site/_ro/trn_rl_repo")):
    if os.path.isdir(_p) and _p not in sys.path:
        sys.path.insert(0, _p)

import numpy as np

import concourse.bass as bass
from concourse import bacc
import concourse.tile as tile
from concourse import mybir
from concourse.masks import make_identity

B, S, H = 64, 2048, 512
N_CORES = 8
BPC = B // N_CORES  # batches per core

FP = mybir.dt.float32
FPR = mybir.dt.float32r
F16 = mybir.dt.float16

GROUP = 512          # seq positions processed per inner group
HC = H // 128        # h chunks of 128


def build_program(bpc=BPC, s=S):
    """Build the per-core Bass program."""
    nc = bacc.Bacc(None)
    n_groups = s // GROUP

    xt = nc.declare_dram_parameter("xt", [H, bpc * s], F16, isOutput=False)
    dh = nc.declare_dram_parameter("dh", [bpc, H], FP, isOutput=False)
    cov = nc.declare_dram_parameter("cov", [bpc, s], FP, isOutput=False)
    cov16 = nc.declare_dram_parameter("cov16", [1, bpc * s], F16, isOutput=False)
    wh = nc.declare_dram_parameter("wh", [H, H], F16, isOutput=False)
    ws = nc.declare_dram_parameter("ws", [H, H], FPR, isOutput=False)
    wc = nc.declare_dram_parameter("wc", [1, H], FP, isOutput=False)
    wc16 = nc.declare_dram_parameter("wc16", [1, H], F16, isOutput=False)
    vw = nc.declare_dram_parameter("vw", [1, H], FP, isOutput=False)
    vw16 = nc.declare_dram_parameter("vw16", [1, H], F16, isOutput=False)
    bh = nc.declare_dram_parameter("bh", [1, H], FPR, isOutput=False)
    bs = nc.declare_dram_parameter("bs", [1, H], FPR, isOutput=False)
    bc = nc.declare_dram_parameter("bc", [1, H], FPR, isOutput=False)
    ones = nc.declare_dram_parameter("ones", [1, bpc], FPR, isOutput=False)
    out_a = nc.declare_dram_parameter("out_a", [bpc, s], FP, isOutput=True)
    out_sc = nc.declare_dram_parameter("out_sc", [bpc, s], FP, isOutput=True)

    from contextlib import ExitStack
    with tile.TileContext(nc) as tc, ExitStack() as ctx:
        const = ctx.enter_context(tc.tile_pool(name="const", bufs=1))
        xtpool = ctx.enter_context(tc.tile_pool(name="xtpool", bufs=12))
        fpool = ctx.enter_context(tc.tile_pool(name="fpool", bufs=4))
        covg_pool = ctx.enter_context(tc.tile_pool(name="covg", bufs=3))
        erow_pool = ctx.enter_context(tc.tile_pool(name="erow", bufs=3))
        ps_f_pool = ctx.enter_context(tc.tile_pool(name="ps_f", bufs=4, space="PSUM"))
        ps_e_pool = ctx.enter_context(tc.tile_pool(name="ps_e", bufs=2, space="PSUM"))
        ps_pre_pool = ctx.enter_context(tc.tile_pool(name="ps_pre", bufs=1, space="PSUM"))

        # ---------------- preamble: constants & params ----------------
        ident = const.tile([128, 128], FP, tag="ident")
        make_identity(nc, ident)

        wh_sb = []
        for k in range(HC):
            t = const.tile([128, H], F16, tag=f"wh{k}", name=f"wh_sb{k}")
            nc.sync.dma_start(out=t, in_=wh[k * 128:(k + 1) * 128, :])
            wh_sb.append(t)
        ws_sb = []
        for k in range(HC):
            t = const.tile([128, H], FPR, tag=f"ws{k}", name=f"ws_sb{k}")
            nc.sync.dma_start(out=t, in_=ws[k * 128:(k + 1) * 128, :])
            ws_sb.append(t)

        wc_sb = const.tile([1, H], FP, tag="wc")
        nc.sync.dma_start(out=wc_sb, in_=wc[:, :])
        wc16_sb = const.tile([1, H], F16, tag="wc16")
        nc.sync.dma_start(out=wc16_sb, in_=wc16[:, :])
        vw_row = const.tile([1, H], FP, tag="vw_row")
        nc.sync.dma_start(out=vw_row, in_=vw[:, :])
        vw16_row = const.tile([1, H], F16, tag="vw16_row")
        nc.sync.dma_start(out=vw16_row, in_=vw16[:, :])
        bh_sb = const.tile([1, H], FPR, tag="bh")
        nc.sync.dma_start(out=bh_sb, in_=bh[:, :])
        bs_sb = const.tile([1, H], FPR, tag="bs")
        nc.sync.dma_start(out=bs_sb, in_=bs[:, :])
        bc_sb = const.tile([1, H], FPR, tag="bc")
        nc.sync.dma_start(out=bc_sb, in_=bc[:, :])
        dh_sb = const.tile([bpc, H], FP, tag="dh")
        nc.sync.dma_start(out=dh_sb, in_=dh[:, :])
        cov_sb = const.tile([bpc, s], FP, tag="cov")
        nc.sync.dma_start(out=cov_sb, in_=cov[:, :])

        # bias sum bh + bs + bc -> [1, H]
        bsum_sb = const.tile([1, H], FPR, tag="bsum")
        nc.vector.tensor_add(bsum_sb, bh_sb, bs_sb)
        nc.vector.tensor_add(bsum_sb, bsum_sb, bc_sb)

        ones_sb = const.tile([1, bpc], FPR, tag="ones")
        nc.sync.dma_start(out=ones_sb, in_=ones[:, :])

        # v_w chunked to [128, HC] (fp16, for the e_t matmuls) and Wc
        # chunked to [128, HC] (fp32, Pool scalar) via PE transpose of
        # [1,128] slices.
        vw_sb = const.tile([128, HC], F16, tag="vw_sb")
        for k in range(HC):
            ps = ps_pre_pool.tile([128, max(bpc, 8)], FP, tag="pre")
            nc.tensor.transpose(
                ps[:, 0:1],
                vw_row[0:1, k * 128:(k + 1) * 128],
                ident[0:1, 0:1],
            )
            nc.vector.tensor_copy(vw_sb[:, k:k + 1], ps[:, 0:1])

        # decoder_hidden transposed: dhT_k [128, bpc]
        dht_sb = []
        for k in range(HC):
            ps = ps_pre_pool.tile([128, max(bpc, 8)], FP, tag="pre")
            nc.tensor.transpose(
                ps[:, 0:bpc],
                dh_sb[0:bpc, k * 128:(k + 1) * 128],
                ident[0:bpc, 0:bpc],
            )
            t = const.tile([128, bpc], FPR, tag=f"dht{k}", name=f"dht{k}")
            nc.vector.tensor_copy(t, ps[:, 0:bpc])
            dht_sb.append(t)

        # A[h, b] = (dh @ Ws).T + (bh + bs + bc) broadcast over b,
        # computed chunk-wise: psA_m = sum_k Ws[k,m].T @ dhT_k + bsum_m.T @ ones
        a_sb = const.tile([128, HC, bpc], FP, tag="a_sb")
        for m in range(HC):
            ms = slice(m * 128, (m + 1) * 128)
            ps = ps_pre_pool.tile([128, max(bpc, 8)], FP, tag="pre")
            for k in range(HC):
                nc.tensor.matmul(
                    ps[:, 0:bpc],
                    ws_sb[k][:, ms],
                    dht_sb[k][:, :],
                    start=(k == 0),
                    stop=False,
                )
            nc.tensor.matmul(
                ps[:, 0:bpc],
                bsum_sb[0:1, ms],
                ones_sb[0:1, :],
                start=False,
                stop=True,
            )
            nc.vector.tensor_copy(a_sb[:, m, :], ps[:, 0:bpc])

        # e_t accumulator [bpc, s]; per-group chunks go through a small
        # partition-0 scratch (DVE writes must start at partition 0) and a
        # SB->SB DMA into row b.
        e_sb = const.tile([bpc, s], FP, tag="e_sb")

        # ---------------- main loop ----------------
        for b in range(bpc):
            for g in range(n_groups):
                gs = slice(g * GROUP, (g + 1) * GROUP)
                c0 = b * s + g * GROUP
                xts = []
                for k in range(HC):
                    xt_k = xtpool.tile([128, GROUP], F16, tag="xt")
                    nc.sync.dma_start(
                        out=xt_k,
                        in_=xt[k * 128:(k + 1) * 128, c0:c0 + GROUP])
                    xts.append(xt_k)
                cov_g = covg_pool.tile([1, GROUP], F16, tag="cov_g")
                nc.sync.dma_start(out=cov_g, in_=cov16[0:1, c0:c0 + GROUP])

                ps_e = ps_e_pool.tile([1, GROUP], FP, tag="ps_e")
                f_ms = []

                def emit_main(m):
                    ms = slice(m * 128, (m + 1) * 128)
                    ps_f = ps_f_pool.tile([128, GROUP], FP, tag="ps_f")
                    for k in range(HC):
                        nc.tensor.matmul(
                            ps_f,
                            wh_sb[k][:, ms],
                            xts[k][:, :],
                            start=(k == 0),
                            stop=False,
                        )
                    # coverage rank-1 term: K=1 fp16 matmul (2 cols/cycle)
                    nc.tensor.matmul(
                        ps_f,
                        wc16_sb[0:1, ms],
                        cov_g[0:1, :],
                        start=False,
                        stop=True,
                    )
                    # tanh with per-partition bias A on ScalarE -> fp16
                    f_m = fpool.tile([128, GROUP], F16, tag="f_m")
                    nc.scalar.activation(
                        out=f_m,
                        in_=ps_f,
                        func=mybir.ActivationFunctionType.Tanh,
                        bias=a_sb[:, m, b:b + 1],
                    )
                    f_ms.append(f_m)

                def emit_et(m):
                    nc.tensor.matmul(
                        ps_e,
                        vw_sb[:, m:m + 1],
                        f_ms[m][:, :],
                        start=(m == 0),
                        stop=(m == HC - 1),
                    )

                emit_main(0)
                emit_main(1)
                emit_et(0)
                emit_main(2)
                emit_et(1)
                emit_main(3)
                emit_et(2)
                emit_et(3)

                e_g = erow_pool.tile([1, GROUP], FP, tag="e_g")
                nc.vector.tensor_copy(e_g, ps_e)
                nc.sync.dma_start(out=e_sb[b:b + 1, gs], in_=e_g)

        # ---------------- softmax + outputs ----------------
        smx = const.tile([bpc, 1], FP, tag="smx")
        nc.vector.tensor_reduce(
            out=smx, in_=e_sb, axis=mybir.AxisListType.X,
            op=mybir.AluOpType.max, negate=True,
        )
        p_sb = const.tile([bpc, s], FP, tag="p_sb")
        esum = const.tile([bpc, 1], FP, tag="esum")
        nc.scalar.activation(
            out=p_sb, in_=e_sb, func=mybir.ActivationFunctionType.Exp,
            bias=smx, accum_out=esum,
        )
        rsum = const.tile([bpc, 1], FP, tag="rsum")
        nc.vector.reciprocal(rsum, esum)
        a_out_sb = const.tile([bpc, s], FP, tag="a_out")
        nc.vector.tensor_scalar_mul(a_out_sb, p_sb, rsum)
        sc_sb = const.tile([bpc, s], FP, tag="sc_sb")
        nc.vector.tensor_add(sc_sb, a_out_sb, cov_sb)
        nc.sync.dma_start(out=out_a[:, :], in_=a_out_sb)
        nc.sync.dma_start(out=out_sc[:, :], in_=sc_sb)

    return nc


_PROG_CACHE = {}


def _get_program(key=(BPC, S)):
    if key not in _PROG_CACHE:
        nc = build_program(*key)
        nc.finalize()
        _PROG_CACHE[key] = nc
    return _PROG_CACHE[key]


def make_in_maps(encoder_output, decoder_hidden, coverage, Wh, bh, Ws, bs, Wc, bc,
                 v_w, v_b=None):
    f32 = np.float32
    enc = np.asarray(encoder_output, dtype=f32)
    dh = np.ascontiguousarray(decoder_hidden, dtype=f32)
    cov = np.ascontiguousarray(coverage, dtype=f32)
    shared = {
        "wh": np.ascontiguousarray(Wh, dtype=np.float16),
        "ws": np.ascontiguousarray(Ws, dtype=f32),
        "wc": np.ascontiguousarray(Wc, dtype=f32).reshape(1, H),
        "wc16": np.ascontiguousarray(Wc, dtype=np.float16).reshape(1, H),
        "vw": np.ascontiguousarray(v_w, dtype=f32).reshape(1, H),
        "vw16": np.ascontiguousarray(v_w, dtype=np.float16).reshape(1, H),
        "bh": np.ascontiguousarray(bh, dtype=f32).reshape(1, H),
        "bs": np.ascontiguousarray(bs, dtype=f32).reshape(1, H),
        "bc": np.ascontiguousarray(bc, dtype=f32).reshape(1, H),
        "ones": np.ones((1, BPC), dtype=f32),
    }
    in_maps = []
    for c in range(N_CORES):
        lo, hi = c * BPC, (c + 1) * BPC
        m = dict(shared)
        # xT [H, bpc*s] fp16: cast + transpose on host so the device
        # streams contiguous fp16 Xt tiles.
        xc = enc[lo:hi].reshape(BPC * S, H).astype(np.float16)
        m["xt"] = np.ascontiguousarray(xc.T)
        m["dh"] = np.ascontiguousarray(dh[lo:hi])
        m["cov"] = np.ascontiguousarray(cov[lo:hi])
        m["cov16"] = m["cov"].reshape(1, BPC * S).astype(np.float16)
        in_maps.append(m)
    return in_maps


def run_spmd(in_maps, trace=False, **kw):
    from concourse.bass_utils import run_bass_kernel_spmd
    nc = _get_program()
    return run_bass_kernel_spmd(nc, in_maps, core_ids=list(range(N_CORES)),
                                trace=trace, **kw)


def kernel(**inputs) -> tuple[np.ndarray, np.ndarray]:
    in_maps = make_in_maps(**inputs)
    res = run_spmd(in_maps)
    a_t = np.concatenate([r["out_a"] for r in res.results], axis=0)
    sum_cov = np.concatenate([r["out_sc"] for r in res.results], axis=0)
    return a_t.astype(np.float32), sum_cov.astype(np.float32)


# revision 6
# speedup vs baseline: 1.3265x; 1.0194x over previous
"""Trainium2 (8 NeuronCores) kernel for coverage attention.

Computes, for inputs (B,S,H)=(64,2048,512):
    enc_f = encoder_output @ Wh + bh            [B,S,H]
    dec_f = decoder_hidden @ Ws + bs            [B,1,H]
    cov_f = coverage[...,None] * Wc[0] + bc     [B,S,H]
    feat  = tanh(enc_f + dec_f + cov_f)
    e_t   = feat @ v_w + v_b                    [B,S]
    a_t   = softmax(e_t, axis=-1)
    sum_coverage = coverage + a_t
returns (a_t, sum_coverage).

Sharding: data-parallel over batch B across 8 cores (8 batches/core).
Params are small and replicated. No collectives needed.

Per-core pipeline (v5):
  - encoder_output is cast to fp16 and pre-transposed on the host into
    xT [H, bpc*s]; the device streams fp16 Xt tiles [128 h, 4 k, 512 s]
    straight from HBM in ONE dma per group (half the f32 traffic, no PE
    transposes, no on-device casts, 4x fewer descriptor-gens).
  - feat.T chunks [h=128, s=512] = Wh_k.T @ Xt_k accumulated in fp32
    PSUM with fp16 operands.
  - DVE drains PSUM while applying the coverage rank-1 term:
    f_pre = cov_b * WcT_m + ps_f (scalar_tensor_tensor), fp16 out;
    cov_b is coverage pre-broadcast to 128 partitions in DRAM.
  - bias A[b,h] = dec_f + bh + bs + bc is applied via the tanh
    activation's per-partition bias; tanh emits fp16 f_m.
  - e_t row chunks [1, 512] = v_w.T @ f_m (K=128, M=1, fp16) for group
    G are interleaved between group G+1's main blocks, so the in-order
    PE queue never waits on the DVE/tanh chain and the PE p-state stays
    at max clock.
  - per-batch e rows go to e_sb [8, 2048] via DVE copy + SB->SB DMA;
    softmax uses free-dim reduce + Exp(bias=-max, accum_out=sum).
  - v_b is omitted: softmax is invariant to constant shifts.
"""

import os
import sys

for _p in ("/opt/trn_rl_repo", os.path.expanduser("~/.axon_site/_ro/trn_rl_repo")):
    if os.path.isdir(_p) and _p not in sys.path:
        sys.path.insert(0, _p)

import numpy as np

import concourse.bass as bass
from concourse import bacc
import concourse.tile as tile
from concourse import mybir
from concourse.masks import make_identity

B, S, H = 64, 2048, 512
N_CORES = 8
BPC = B // N_CORES  # batches per core

FP = mybir.dt.float32
FPR = mybir.dt.float32r
F16 = mybir.dt.float16

GROUP = 512          # seq positions processed per inner group
HC = H // 128        # h chunks of 128


def build_program(bpc=BPC, s=S):
    """Build the per-core Bass program."""
    nc = bacc.Bacc(None)
    n_groups = s // GROUP

    xt = nc.declare_dram_parameter("xt", [H, bpc * s], F16, isOutput=False)
    dh = nc.declare_dram_parameter("dh", [bpc, H], FP, isOutput=False)
    cov = nc.declare_dram_parameter("cov", [bpc, s], FP, isOutput=False)
    covb = nc.declare_dram_parameter("covb", [128, bpc * s], FP, isOutput=False)
    wh = nc.declare_dram_parameter("wh", [H, H], F16, isOutput=False)
    ws = nc.declare_dram_parameter("ws", [H, H], FPR, isOutput=False)
    wc = nc.declare_dram_parameter("wc", [1, H], FP, isOutput=False)
    vw = nc.declare_dram_parameter("vw", [1, H], FP, isOutput=False)
    vw16 = nc.declare_dram_parameter("vw16", [1, H], F16, isOutput=False)
    bh = nc.declare_dram_parameter("bh", [1, H], FPR, isOutput=False)
    bs = nc.declare_dram_parameter("bs", [1, H], FPR, isOutput=False)
    bc = nc.declare_dram_parameter("bc", [1, H], FPR, isOutput=False)
    ones = nc.declare_dram_parameter("ones", [1, bpc], FPR, isOutput=False)
    out_a = nc.declare_dram_parameter("out_a", [bpc, s], FP, isOutput=True)
    out_sc = nc.declare_dram_parameter("out_sc", [bpc, s], FP, isOutput=True)

    from contextlib import ExitStack
    with tile.TileContext(nc) as tc, ExitStack() as ctx:
        const = ctx.enter_context(tc.tile_pool(name="const", bufs=1))
        xtpool = ctx.enter_context(tc.tile_pool(name="xtpool", bufs=3))
        fpool = ctx.enter_context(tc.tile_pool(name="fpool", bufs=9))
        fprepool = ctx.enter_context(tc.tile_pool(name="fpre", bufs=4))
        covb_pool = ctx.enter_context(tc.tile_pool(name="covb", bufs=3))
        erow_pool = ctx.enter_context(tc.tile_pool(name="erow", bufs=3))
        ps_f_pool = ctx.enter_context(tc.tile_pool(name="ps_f", bufs=4, space="PSUM"))
        ps_e_pool = ctx.enter_context(tc.tile_pool(name="ps_e", bufs=3, space="PSUM"))
        ps_pre_pool = ctx.enter_context(tc.tile_pool(name="ps_pre", bufs=1, space="PSUM"))

        # ---------------- preamble: constants & params ----------------
        ident = const.tile([128, 128], FP, tag="ident")
        make_identity(nc, ident)

        wh_sb = []
        for k in range(HC):
            t = const.tile([128, H], F16, tag=f"wh{k}", name=f"wh_sb{k}")
            nc.sync.dma_start(out=t, in_=wh[k * 128:(k + 1) * 128, :])
            wh_sb.append(t)
        ws_sb = []
        for k in range(HC):
            t = const.tile([128, H], FPR, tag=f"ws{k}", name=f"ws_sb{k}")
            nc.sync.dma_start(out=t, in_=ws[k * 128:(k + 1) * 128, :])
            ws_sb.append(t)

        wc_sb = const.tile([1, H], FP, tag="wc")
        nc.sync.dma_start(out=wc_sb, in_=wc[:, :])
        vw_row = const.tile([1, H], FP, tag="vw_row")
        nc.sync.dma_start(out=vw_row, in_=vw[:, :])
        vw16_row = const.tile([1, H], F16, tag="vw16_row")
        nc.sync.dma_start(out=vw16_row, in_=vw16[:, :])
        bh_sb = const.tile([1, H], FPR, tag="bh")
        nc.sync.dma_start(out=bh_sb, in_=bh[:, :])
        bs_sb = const.tile([1, H], FPR, tag="bs")
        nc.sync.dma_start(out=bs_sb, in_=bs[:, :])
        bc_sb = const.tile([1, H], FPR, tag="bc")
        nc.sync.dma_start(out=bc_sb, in_=bc[:, :])
        dh_sb = const.tile([bpc, H], FP, tag="dh")
        nc.sync.dma_start(out=dh_sb, in_=dh[:, :])
        cov_sb = const.tile([bpc, s], FP, tag="cov")
        nc.sync.dma_start(out=cov_sb, in_=cov[:, :])

        # bias sum bh + bs + bc -> [1, H]
        bsum_sb = const.tile([1, H], FPR, tag="bsum")
        nc.vector.tensor_add(bsum_sb, bh_sb, bs_sb)
        nc.vector.tensor_add(bsum_sb, bsum_sb, bc_sb)

        ones_sb = const.tile([1, bpc], FPR, tag="ones")
        nc.sync.dma_start(out=ones_sb, in_=ones[:, :])

        # v_w chunked to [128, HC] (fp16, e_t matmuls) and Wc chunked to
        # [128, HC] (fp32, DVE scalar) via PE transpose of [1,128] slices
        vw_sb = const.tile([128, HC], F16, tag="vw_sb")
        wct_sb = const.tile([128, HC], FP, tag="wct_sb")
        for k in range(HC):
            ps = ps_pre_pool.tile([128, max(bpc, 8)], FP, tag="pre")
            nc.tensor.transpose(
                ps[:, 0:1],
                vw_row[0:1, k * 128:(k + 1) * 128],
                ident[0:1, 0:1],
            )
            nc.vector.tensor_copy(vw_sb[:, k:k + 1], ps[:, 0:1])
            ps2 = ps_pre_pool.tile([128, max(bpc, 8)], FP, tag="pre")
            nc.tensor.transpose(
                ps2[:, 0:1],
                wc_sb[0:1, k * 128:(k + 1) * 128],
                ident[0:1, 0:1],
            )
            nc.vector.tensor_copy(wct_sb[:, k:k + 1], ps2[:, 0:1])

        # decoder_hidden transposed: dhT_k [128, bpc]
        dht_sb = []
        for k in range(HC):
            ps = ps_pre_pool.tile([128, max(bpc, 8)], FP, tag="pre")
            nc.tensor.transpose(
                ps[:, 0:bpc],
                dh_sb[0:bpc, k * 128:(k + 1) * 128],
                ident[0:bpc, 0:bpc],
            )
            t = const.tile([128, bpc], FPR, tag=f"dht{k}", name=f"dht{k}")
            nc.vector.tensor_copy(t, ps[:, 0:bpc])
            dht_sb.append(t)

        # A[h, b] = (dh @ Ws).T + (bh + bs + bc) broadcast over b,
        # computed chunk-wise: psA_m = sum_k Ws[k,m].T @ dhT_k + bsum_m.T @ ones
        a_sb = const.tile([128, HC, bpc], FP, tag="a_sb")
        for m in range(HC):
            ms = slice(m * 128, (m + 1) * 128)
            ps = ps_pre_pool.tile([128, max(bpc, 8)], FP, tag="pre")
            for k in range(HC):
                nc.tensor.matmul(
                    ps[:, 0:bpc],
                    ws_sb[k][:, ms],
                    dht_sb[k][:, :],
                    start=(k == 0),
                    stop=False,
                )
            nc.tensor.matmul(
                ps[:, 0:bpc],
                bsum_sb[0:1, ms],
                ones_sb[0:1, :],
                start=False,
                stop=True,
            )
            nc.vector.tensor_copy(a_sb[:, m, :], ps[:, 0:bpc])

        # e_t accumulator [bpc, s]
        e_sb = const.tile([bpc, s], FP, tag="e_sb")

        # ---------------- main loop ----------------
        # Software pipeline across groups: group G's e_t matmuls are
        # emitted between group G+1's main blocks.
        groups = [(b, g) for b in range(bpc) for g in range(n_groups)]
        prev = None  # (b, g, f_ms, ps_e)

        def emit_et(prev_state, m):
            _b, _g, f_ms, ps_e = prev_state
            nc.tensor.matmul(
                ps_e,
                vw_sb[:, m:m + 1],
                f_ms[m][:, :],
                start=(m == 0),
                stop=(m == HC - 1),
            )

        def drain_e(prev_state):
            _b, _g, _f, ps_e = prev_state
            e_g = erow_pool.tile([1, GROUP], FP, tag="e_g")
            nc.vector.tensor_copy(e_g, ps_e)
            nc.sync.dma_start(
                out=e_sb[_b:_b + 1, _g * GROUP:(_g + 1) * GROUP], in_=e_g)

        for (b, g) in groups:
            c0 = b * s + g * GROUP
            xt_all = xtpool.tile([128, HC, GROUP], F16, tag="xt")
            nc.sync.dma_start(
                out=xt_all,
                in_=xt[:, c0:c0 + GROUP].rearrange("(k p) n -> p k n", p=128))
            cov_b = covb_pool.tile([128, GROUP], FP, tag="cov_b")
            nc.sync.dma_start(out=cov_b, in_=covb[:, c0:c0 + GROUP])

            ps_e = ps_e_pool.tile([1, GROUP], FP, tag="ps_e")
            f_ms = []
            for m in range(HC):
                ms = slice(m * 128, (m + 1) * 128)
                ps_f = ps_f_pool.tile([128, GROUP], FP, tag="ps_f")
                for k in range(HC):
                    nc.tensor.matmul(
                        ps_f,
                        wh_sb[k][:, ms],
                        xt_all[:, k, :],
                        start=(k == 0),
                        stop=(k == HC - 1),
                    )
                # DVE drains PSUM + coverage term: f_pre = cov_b*WcT_m + ps_f
                f_pre = fprepool.tile([128, GROUP], F16, tag="f_pre")
                nc.vector.scalar_tensor_tensor(
                    out=f_pre,
                    in0=cov_b,
                    scalar=wct_sb[:, m:m + 1],
                    in1=ps_f,
                    op0=mybir.AluOpType.mult,
                    op1=mybir.AluOpType.add,
                )
                # tanh with per-partition bias A on ScalarE -> fp16
                f_m = fpool.tile([128, GROUP], F16, tag="f_m")
                nc.scalar.activation(
                    out=f_m,
                    in_=f_pre,
                    func=mybir.ActivationFunctionType.Tanh,
                    bias=a_sb[:, m, b:b + 1],
                )
                f_ms.append(f_m)
                if prev is not None:
                    emit_et(prev, m)
            if prev is not None:
                drain_e(prev)
            prev = (b, g, f_ms, ps_e)

        for m in range(HC):
            emit_et(prev, m)
        drain_e(prev)

        # ---------------- softmax + outputs ----------------
        smx = const.tile([bpc, 1], FP, tag="smx")
        nc.vector.tensor_reduce(
            out=smx, in_=e_sb, axis=mybir.AxisListType.X,
            op=mybir.AluOpType.max, negate=True,
        )
        p_sb = const.tile([bpc, s], FP, tag="p_sb")
        esum = const.tile([bpc, 1], FP, tag="esum")
        nc.scalar.activation(
            out=p_sb, in_=e_sb, func=mybir.ActivationFunctionType.Exp,
            bias=smx, accum_out=esum,
        )
        rsum = const.tile([bpc, 1], FP, tag="rsum")
        nc.vector.reciprocal(rsum, esum)
        a_out_sb = const.tile([bpc, s], FP, tag="a_out")
        nc.vector.tensor_scalar_mul(a_out_sb, p_sb, rsum)
        sc_sb = const.tile([bpc, s], FP, tag="sc_sb")
        nc.vector.tensor_add(sc_sb, a_out_sb, cov_sb)
        nc.sync.dma_start(out=out_a[:, :], in_=a_out_sb)
        nc.sync.dma_start(out=out_sc[:, :], in_=sc_sb)

    return nc


_PROG_CACHE = {}


def _get_program(key=(BPC, S)):
    if key not in _PROG_CACHE:
        nc = build_program(*key)
        nc.finalize()
        _PROG_CACHE[key] = nc
    return _PROG_CACHE[key]


def make_in_maps(encoder_output, decoder_hidden, coverage, Wh, bh, Ws, bs, Wc, bc,
                 v_w, v_b=None):
    f32 = np.float32
    enc = np.asarray(encoder_output, dtype=f32)
    dh = np.ascontiguousarray(decoder_hidden, dtype=f32)
    cov = np.ascontiguousarray(coverage, dtype=f32)
    shared = {
        "wh": np.ascontiguousarray(Wh, dtype=np.float16),
        "ws": np.ascontiguousarray(Ws, dtype=f32),
        "wc": np.ascontiguousarray(Wc, dtype=f32).reshape(1, H),
        "vw": np.ascontiguousarray(v_w, dtype=f32).reshape(1, H),
        "vw16": np.ascontiguousarray(v_w, dtype=np.float16).reshape(1, H),
        "bh": np.ascontiguousarray(bh, dtype=f32).reshape(1, H),
        "bs": np.ascontiguousarray(bs, dtype=f32).reshape(1, H),
        "bc": np.ascontiguousarray(bc, dtype=f32).reshape(1, H),
        "ones": np.ones((1, BPC), dtype=f32),
    }
    in_maps = []
    for c in range(N_CORES):
        lo, hi = c * BPC, (c + 1) * BPC
        m = dict(shared)
        # xT [H, bpc*s] fp16: cast + transpose on host so the device
        # streams contiguous fp16 Xt tiles.
        xc = enc[lo:hi].reshape(BPC * S, H).astype(np.float16)
        m["xt"] = np.ascontiguousarray(xc.T)
        m["dh"] = np.ascontiguousarray(dh[lo:hi])
        m["cov"] = np.ascontiguousarray(cov[lo:hi])
        m["covb"] = np.ascontiguousarray(
            np.broadcast_to(m["cov"].reshape(1, BPC * S), (128, BPC * S)))
        in_maps.append(m)
    return in_maps


def run_spmd(in_maps, trace=False, **kw):
    from concourse.bass_utils import run_bass_kernel_spmd
    nc = _get_program()
    return run_bass_kernel_spmd(nc, in_maps, core_ids=list(range(N_CORES)),
                                trace=trace, **kw)


def kernel(**inputs) -> tuple[np.ndarray, np.ndarray]:
    in_maps = make_in_maps(**inputs)
    res = run_spmd(in_maps)
    a_t = np.concatenate([r["out_a"] for r in res.results], axis=0)
    sum_cov = np.concatenate([r["out_sc"] for r in res.results], axis=0)
    return a_t.astype(np.float32), sum_cov.astype(np.float32)


# revision 7
# speedup vs baseline: 1.6988x; 1.2806x over previous
"""Trainium2 (8 NeuronCores) kernel for coverage attention.

Computes, for inputs (B,S,H)=(64,2048,512):
    enc_f = encoder_output @ Wh + bh            [B,S,H]
    dec_f = decoder_hidden @ Ws + bs            [B,1,H]
    cov_f = coverage[...,None] * Wc[0] + bc     [B,S,H]
    feat  = tanh(enc_f + dec_f + cov_f)
    e_t   = feat @ v_w + v_b                    [B,S]
    a_t   = softmax(e_t, axis=-1)
    sum_coverage = coverage + a_t
returns (a_t, sum_coverage).

Sharding: data-parallel over batch B across 8 cores (8 batches/core).
Params are small and replicated. No collectives needed.

Per-core pipeline (v5):
  - encoder_output is cast to fp16 and pre-transposed on the host into
    xT [H, bpc*s]; the device streams fp16 Xt tiles [128 h, 4 k, 512 s]
    straight from HBM in ONE dma per group (half the f32 traffic, no PE
    transposes, no on-device casts, 4x fewer descriptor-gens).
  - feat.T chunks [h=128, s=512] = Wh_k.T @ Xt_k accumulated in fp32
    PSUM with fp16 operands.
  - DVE drains PSUM while applying the coverage rank-1 term:
    f_pre = cov_b * WcT_m + ps_f (scalar_tensor_tensor), fp16 out;
    cov_b is coverage pre-broadcast to 128 partitions in DRAM.
  - bias A[b,h] = dec_f + bh + bs + bc is applied via the tanh
    activation's per-partition bias; tanh emits fp16 f_m.
  - e_t row chunks [1, 512] = v_w.T @ f_m (K=128, M=1, fp16) for group
    G run as one 4-matmul batch in the middle of group G+1's mains, so
    the in-order PE queue never waits on the DVE/tanh chain and the
    LDWEIGHTS-exposure penalty is paid once per group, not 4 times.
  - per-batch e rows go to e_sb [8, 2048] via DVE copy + SB->SB DMA;
    softmax uses free-dim reduce + Exp(bias=-max, accum_out=sum).
  - v_b is omitted: softmax is invariant to constant shifts.
"""

import os
import sys

for _p in ("/opt/trn_rl_repo", os.path.expanduser("~/.axon_site/_ro/trn_rl_repo")):
    if os.path.isdir(_p) and _p not in sys.path:
        sys.path.insert(0, _p)

import numpy as np

import concourse.bass as bass
from concourse import bacc
import concourse.tile as tile
from concourse import mybir
from concourse.masks import make_identity

B, S, H = 64, 2048, 512
N_CORES = 8
BPC = B // N_CORES  # batches per core

FP = mybir.dt.float32
FPR = mybir.dt.float32r
F16 = mybir.dt.float16

GROUP = 512          # seq positions processed per inner group
HC = H // 128        # h chunks of 128


def build_program(bpc=BPC, s=S):
    """Build the per-core Bass program."""
    nc = bacc.Bacc(None)
    n_groups = s // GROUP

    xt = nc.declare_dram_parameter("xt", [H, bpc * s], F16, isOutput=False)
    dh = nc.declare_dram_parameter("dh", [bpc, H], FP, isOutput=False)
    cov = nc.declare_dram_parameter("cov", [bpc, s], FP, isOutput=False)
    covb = nc.declare_dram_parameter("covb", [128, bpc * s], FP, isOutput=False)
    wh = nc.declare_dram_parameter("wh", [H, H], F16, isOutput=False)
    ws = nc.declare_dram_parameter("ws", [H, H], FPR, isOutput=False)
    wc = nc.declare_dram_parameter("wc", [1, H], FP, isOutput=False)
    vw = nc.declare_dram_parameter("vw", [1, H], FP, isOutput=False)
    vw16 = nc.declare_dram_parameter("vw16", [1, H], F16, isOutput=False)
    bh = nc.declare_dram_parameter("bh", [1, H], FPR, isOutput=False)
    bs = nc.declare_dram_parameter("bs", [1, H], FPR, isOutput=False)
    bc = nc.declare_dram_parameter("bc", [1, H], FPR, isOutput=False)
    ones = nc.declare_dram_parameter("ones", [1, bpc], FPR, isOutput=False)
    out_a = nc.declare_dram_parameter("out_a", [bpc, s], FP, isOutput=True)
    out_sc = nc.declare_dram_parameter("out_sc", [bpc, s], FP, isOutput=True)

    from contextlib import ExitStack
    with tile.TileContext(nc) as tc, ExitStack() as ctx:
        const = ctx.enter_context(tc.tile_pool(name="const", bufs=1))
        xtpool = ctx.enter_context(tc.tile_pool(name="xtpool", bufs=3))
        fpool = ctx.enter_context(tc.tile_pool(name="fpool", bufs=9))
        fprepool = ctx.enter_context(tc.tile_pool(name="fpre", bufs=4))
        covb_pool = ctx.enter_context(tc.tile_pool(name="covb", bufs=3))
        erow_pool = ctx.enter_context(tc.tile_pool(name="erow", bufs=3))
        ps_f_pool = ctx.enter_context(tc.tile_pool(name="ps_f", bufs=4, space="PSUM"))
        ps_e_pool = ctx.enter_context(tc.tile_pool(name="ps_e", bufs=3, space="PSUM"))
        ps_pre_pool = ctx.enter_context(tc.tile_pool(name="ps_pre", bufs=1, space="PSUM"))

        # ---------------- preamble: constants & params ----------------
        ident = const.tile([128, 128], FP, tag="ident")
        make_identity(nc, ident)

        wh_sb = []
        for k in range(HC):
            t = const.tile([128, H], F16, tag=f"wh{k}", name=f"wh_sb{k}")
            nc.sync.dma_start(out=t, in_=wh[k * 128:(k + 1) * 128, :])
            wh_sb.append(t)
        ws_sb = []
        for k in range(HC):
            t = const.tile([128, H], FPR, tag=f"ws{k}", name=f"ws_sb{k}")
            nc.sync.dma_start(out=t, in_=ws[k * 128:(k + 1) * 128, :])
            ws_sb.append(t)

        wc_sb = const.tile([1, H], FP, tag="wc")
        nc.sync.dma_start(out=wc_sb, in_=wc[:, :])
        vw_row = const.tile([1, H], FP, tag="vw_row")
        nc.sync.dma_start(out=vw_row, in_=vw[:, :])
        vw16_row = const.tile([1, H], F16, tag="vw16_row")
        nc.sync.dma_start(out=vw16_row, in_=vw16[:, :])
        bh_sb = const.tile([1, H], FPR, tag="bh")
        nc.sync.dma_start(out=bh_sb, in_=bh[:, :])
        bs_sb = const.tile([1, H], FPR, tag="bs")
        nc.sync.dma_start(out=bs_sb, in_=bs[:, :])
        bc_sb = const.tile([1, H], FPR, tag="bc")
        nc.sync.dma_start(out=bc_sb, in_=bc[:, :])
        dh_sb = const.tile([bpc, H], FP, tag="dh")
        nc.sync.dma_start(out=dh_sb, in_=dh[:, :])
        cov_sb = const.tile([bpc, s], FP, tag="cov")
        nc.sync.dma_start(out=cov_sb, in_=cov[:, :])

        # bias sum bh + bs + bc -> [1, H]
        bsum_sb = const.tile([1, H], FPR, tag="bsum")
        nc.vector.tensor_add(bsum_sb, bh_sb, bs_sb)
        nc.vector.tensor_add(bsum_sb, bsum_sb, bc_sb)

        ones_sb = const.tile([1, bpc], FPR, tag="ones")
        nc.sync.dma_start(out=ones_sb, in_=ones[:, :])

        # v_w chunked to [128, HC] (fp16, e_t matmuls) and Wc chunked to
        # [128, HC] (fp32, DVE scalar) via PE transpose of [1,128] slices
        vw_sb = const.tile([128, HC], F16, tag="vw_sb")
        wct_sb = const.tile([128, HC], FP, tag="wct_sb")
        for k in range(HC):
            ps = ps_pre_pool.tile([128, max(bpc, 8)], FP, tag="pre")
            nc.tensor.transpose(
                ps[:, 0:1],
                vw_row[0:1, k * 128:(k + 1) * 128],
                ident[0:1, 0:1],
            )
            nc.vector.tensor_copy(vw_sb[:, k:k + 1], ps[:, 0:1])
            ps2 = ps_pre_pool.tile([128, max(bpc, 8)], FP, tag="pre")
            nc.tensor.transpose(
                ps2[:, 0:1],
                wc_sb[0:1, k * 128:(k + 1) * 128],
                ident[0:1, 0:1],
            )
            nc.vector.tensor_copy(wct_sb[:, k:k + 1], ps2[:, 0:1])

        # decoder_hidden transposed: dhT_k [128, bpc]
        dht_sb = []
        for k in range(HC):
            ps = ps_pre_pool.tile([128, max(bpc, 8)], FP, tag="pre")
            nc.tensor.transpose(
                ps[:, 0:bpc],
                dh_sb[0:bpc, k * 128:(k + 1) * 128],
                ident[0:bpc, 0:bpc],
            )
            t = const.tile([128, bpc], FPR, tag=f"dht{k}", name=f"dht{k}")
            nc.vector.tensor_copy(t, ps[:, 0:bpc])
            dht_sb.append(t)

        # A[h, b] = (dh @ Ws).T + (bh + bs + bc) broadcast over b,
        # computed chunk-wise: psA_m = sum_k Ws[k,m].T @ dhT_k + bsum_m.T @ ones
        a_sb = const.tile([128, HC, bpc], FP, tag="a_sb")
        for m in range(HC):
            ms = slice(m * 128, (m + 1) * 128)
            ps = ps_pre_pool.tile([128, max(bpc, 8)], FP, tag="pre")
            for k in range(HC):
                nc.tensor.matmul(
                    ps[:, 0:bpc],
                    ws_sb[k][:, ms],
                    dht_sb[k][:, :],
                    start=(k == 0),
                    stop=False,
                )
            nc.tensor.matmul(
                ps[:, 0:bpc],
                bsum_sb[0:1, ms],
                ones_sb[0:1, :],
                start=False,
                stop=True,
            )
            nc.vector.tensor_copy(a_sb[:, m, :], ps[:, 0:bpc])

        # e_t accumulator [bpc, s]
        e_sb = const.tile([bpc, s], FP, tag="e_sb")

        # ---------------- main loop ----------------
        # Software pipeline across groups: group G's e_t matmuls are
        # emitted between group G+1's main blocks.
        groups = [(b, g) for b in range(bpc) for g in range(n_groups)]
        prev = None  # (b, g, f_ms, ps_e)

        def emit_et(prev_state, m):
            _b, _g, f_ms, ps_e = prev_state
            nc.tensor.matmul(
                ps_e,
                vw_sb[:, m:m + 1],
                f_ms[m][:, :],
                start=(m == 0),
                stop=(m == HC - 1),
            )

        def drain_e(prev_state):
            _b, _g, _f, ps_e = prev_state
            e_g = erow_pool.tile([1, GROUP], FP, tag="e_g")
            nc.scalar.activation(
                out=e_g, in_=ps_e,
                func=mybir.ActivationFunctionType.Copy)
            nc.sync.dma_start(
                out=e_sb[_b:_b + 1, _g * GROUP:(_g + 1) * GROUP], in_=e_g)

        for (b, g) in groups:
            c0 = b * s + g * GROUP
            xt_all = xtpool.tile([128, HC, GROUP], F16, tag="xt")
            nc.sync.dma_start(
                out=xt_all,
                in_=xt[:, c0:c0 + GROUP].rearrange("(k p) n -> p k n", p=128))
            cov_b = covb_pool.tile([128, GROUP], FP, tag="cov_b")
            nc.sync.dma_start(out=cov_b, in_=covb[:, c0:c0 + GROUP])

            ps_e = ps_e_pool.tile([1, GROUP], FP, tag="ps_e")
            f_ms = []
            for m in range(HC):
                ms = slice(m * 128, (m + 1) * 128)
                ps_f = ps_f_pool.tile([128, GROUP], FP, tag="ps_f")
                for k in range(HC):
                    nc.tensor.matmul(
                        ps_f,
                        wh_sb[k][:, ms],
                        xt_all[:, k, :],
                        start=(k == 0),
                        stop=(k == HC - 1),
                    )
                # DVE drains PSUM + coverage term: f_pre = cov_b*WcT_m + ps_f
                f_pre = fprepool.tile([128, GROUP], F16, tag="f_pre")
                nc.vector.scalar_tensor_tensor(
                    out=f_pre,
                    in0=cov_b,
                    scalar=wct_sb[:, m:m + 1],
                    in1=ps_f,
                    op0=mybir.AluOpType.mult,
                    op1=mybir.AluOpType.add,
                )
                # tanh with per-partition bias A on ScalarE -> fp16
                f_m = fpool.tile([128, GROUP], F16, tag="f_m")
                nc.scalar.activation(
                    out=f_m,
                    in_=f_pre,
                    func=mybir.ActivationFunctionType.Tanh,
                    bias=a_sb[:, m, b:b + 1],
                )
                f_ms.append(f_m)
                if m == 1 and prev is not None:
                    for pm in range(HC):
                        emit_et(prev, pm)
                    drain_e(prev)
            prev = (b, g, f_ms, ps_e)

        for m in range(HC):
            emit_et(prev, m)
        drain_e(prev)

        # ---------------- softmax + outputs ----------------
        smx = const.tile([bpc, 1], FP, tag="smx")
        nc.vector.tensor_reduce(
            out=smx, in_=e_sb, axis=mybir.AxisListType.X,
            op=mybir.AluOpType.max, negate=True,
        )
        p_sb = const.tile([bpc, s], FP, tag="p_sb")
        esum = const.tile([bpc, 1], FP, tag="esum")
        nc.scalar.activation(
            out=p_sb, in_=e_sb, func=mybir.ActivationFunctionType.Exp,
            bias=smx, accum_out=esum,
        )
        rsum = const.tile([bpc, 1], FP, tag="rsum")
        nc.vector.reciprocal(rsum, esum)
        a_out_sb = const.tile([bpc, s], FP, tag="a_out")
        nc.vector.tensor_scalar_mul(a_out_sb, p_sb, rsum)
        sc_sb = const.tile([bpc, s], FP, tag="sc_sb")
        nc.vector.tensor_add(sc_sb, a_out_sb, cov_sb)
        nc.sync.dma_start(out=out_a[:, :], in_=a_out_sb)
        nc.sync.dma_start(out=out_sc[:, :], in_=sc_sb)

    return nc


_PROG_CACHE = {}


def _get_program(key=(BPC, S)):
    if key not in _PROG_CACHE:
        nc = build_program(*key)
        nc.finalize()
        _PROG_CACHE[key] = nc
    return _PROG_CACHE[key]


def make_in_maps(encoder_output, decoder_hidden, coverage, Wh, bh, Ws, bs, Wc, bc,
                 v_w, v_b=None):
    f32 = np.float32
    enc = np.asarray(encoder_output, dtype=f32)
    dh = np.ascontiguousarray(decoder_hidden, dtype=f32)
    cov = np.ascontiguousarray(coverage, dtype=f32)
    shared = {
        "wh": np.ascontiguousarray(Wh, dtype=np.float16),
        "ws": np.ascontiguousarray(Ws, dtype=f32),
        "wc": np.ascontiguousarray(Wc, dtype=f32).reshape(1, H),
        "vw": np.ascontiguousarray(v_w, dtype=f32).reshape(1, H),
        "vw16": np.ascontiguousarray(v_w, dtype=np.float16).reshape(1, H),
        "bh": np.ascontiguousarray(bh, dtype=f32).reshape(1, H),
        "bs": np.ascontiguousarray(bs, dtype=f32).reshape(1, H),
        "bc": np.ascontiguousarray(bc, dtype=f32).reshape(1, H),
        "ones": np.ones((1, BPC), dtype=f32),
    }
    in_maps = []
    for c in range(N_CORES):
        lo, hi = c * BPC, (c + 1) * BPC
        m = dict(shared)
        # xT [H, bpc*s] fp16: cast + transpose on host so the device
        # streams contiguous fp16 Xt tiles.
        xc = enc[lo:hi].reshape(BPC * S, H).astype(np.float16)
        m["xt"] = np.ascontiguousarray(xc.T)
        m["dh"] = np.ascontiguousarray(dh[lo:hi])
        m["cov"] = np.ascontiguousarray(cov[lo:hi])
        m["covb"] = np.ascontiguousarray(
            np.broadcast_to(m["cov"].reshape(1, BPC * S), (128, BPC * S)))
        in_maps.append(m)
    return in_maps


def run_spmd(in_maps, trace=False, **kw):
    from concourse.bass_utils import run_bass_kernel_spmd
    nc = _get_program()
    return run_bass_kernel_spmd(nc, in_maps, core_ids=list(range(N_CORES)),
                                trace=trace, **kw)


def kernel(**inputs) -> tuple[np.ndarray, np.ndarray]:
    in_maps = make_in_maps(**inputs)
    res = run_spmd(in_maps)
    a_t = np.concatenate([r["out_a"] for r in res.results], axis=0)
    sum_cov = np.concatenate([r["out_sc"] for r in res.results], axis=0)
    return a_t.astype(np.float32), sum_cov.astype(np.float32)
